# revision 1
# baseline (speedup 1.0000x reference)
"""Trainium2 Bass kernel for nn_Attention_67765993996325.

Attention with 2D relative position bias:
  qkv = w_qkv @ x_flat ; q *= 512
  sim[i,j] = q_i . k_j + q_i . rel_h[y_j - y_i + 31] + q_i . rel_w[x_j - x_i + 31]
  out = softmax(sim) @ v

Sharding: batch (8) -> one NeuronCore each (data parallel); all 8 heads per core.

Per-core algorithm (all on-chip, B=1, heads=8, n=1024, d=64):
  The relative-position bias and the softmax max-shift are folded into the
  attention matmul itself via augmented contraction channels (K=128):
    channels  0..63  : q[d,i]           (moving)  x  k[d,j]          (stationary)
    channels 64..95  : BH_T[y,i]-m_i    (moving)  x  [y_j==y]        (stationary)
    channels 96..127 : BW_T[x,i]        (moving)  x  [x_j==x]        (stationary)
  where BH_T[y,i] = q_i . rel_h[y - y_i + 31]  (computed by 32 small matmuls
  against pre-shifted slices of rel_h^T), likewise BW_T with rel_w.
  One K=128 matmul then yields sim + bias - m directly; a row max m comes from
  a cheap bf16 stats pass in the normal [i,j] layout (max is shift/precision
  tolerant), and the main pass runs transposed [j,i] so exp(psum) output feeds
  the attention*V matmul with no transposes of the attention matrix at all.
  A ones-column appended to V yields the softmax denominators in the same
  matmul's extra output row.
"""

import os
import sys

sys.path.insert(0, "/opt/trn_rl_repo")

import numpy as np

NUM_HEADS = 8
DIM_HEAD = 64
SCALE = 512.0
B, C, H, W = 8, 64, 32, 32
N = H * W  # 1024

_cache = {}


def _build_program():
    import concourse.bass as bass
    import concourse.mybir as mybir
    import concourse.tile as tile
    from concourse import bacc

    f32 = mybir.dt.float32
    bf16 = mybir.dt.bfloat16
    fp16 = mybir.dt.float16

    nc = bacc.Bacc(None, target_bir_lowering=False)

    xf_p = nc.declare_dram_parameter("xf", [C, N], f32, isOutput=False)
    wT_p = nc.declare_dram_parameter("wT", [C, 3 * NUM_HEADS * DIM_HEAD], f32, isOutput=False)
    relcat_p = nc.declare_dram_parameter("relcat", [64, 32 * 64], f32, isOutput=False)
    oh_p = nc.declare_dram_parameter("oh", [64, N], f32, isOutput=False)
    out_p = nc.declare_dram_parameter("out", [NUM_HEADS * DIM_HEAD, N], f32, isOutput=True)

    HN = NUM_HEADS * N  # 8192

    with tile.TileContext(nc) as tc:
        with tc.tile_pool(name="persist", bufs=1) as persist, \
             tc.tile_pool(name="dram", bufs=2, space="DRAM") as dram:

            # ---- persistent SBUF tensors ----
            qc = persist.tile([128, HN], f32)    # combined moving operand
            kc = persist.tile([128, HN], f32)    # combined stationary operand
            qcb = persist.tile([128, HN], bf16)  # bf16 copy for stats
            kcb = persist.tile([128, HN], bf16)
            v_all = persist.tile([128, 8 * 520], fp16)  # per j-tile: 8 heads x (64 v + 1 ones)
            negm_all = persist.tile([128, 64], f32)     # -rowmax per (head, i-tile)

            # one-hot channels -> kc rows 64..127, replicated per head
            for h in range(NUM_HEADS):
                nc.sync.dma_start(out=kc[64:128, h * N:(h + 1) * N], in_=oh_p[:, :])

            # ---- setup phase: projections + bias tables ----
            with tc.tile_pool(name="setup_ps", bufs=1, space="PSUM") as setup_ps, \
                 tc.tile_pool(name="setup_sb", bufs=1) as setup_sb:
                xf = setup_sb.tile([C, N], f32)
                wT = setup_sb.tile([C, 3 * NUM_HEADS * DIM_HEAD], f32)
                relcat = setup_sb.tile([64, 32 * 64], f32)
                nc.sync.dma_start(out=xf, in_=xf_p[:, :])
                nc.sync.dma_start(out=wT, in_=wT_p[:, :])
                nc.sync.dma_start(out=relcat, in_=relcat_p[:, :])
                # q, k projections: M=64 tiles so copies never shift partitions
                for ot in range(16):
                    pp = setup_ps.tile([64, N], f32, name=f"pp{ot}", tag="pp", bufs=2)
                    nc.tensor.matmul(pp[:, 0:512], wT[:, ot * 64:(ot + 1) * 64],
                                     xf[:, 0:512], start=True, stop=True)
                    nc.tensor.matmul(pp[:, 512:1024], wT[:, ot * 64:(ot + 1) * 64],
                                     xf[:, 512:1024], start=True, stop=True)
                    dst = qc if ot < 8 else kc
                    h0 = ot % 8
                    nc.scalar.copy(out=dst[0:64, h0 * N:(h0 + 1) * N], in_=pp[:, :])

                # v projection, transposed: v_T[n, (h d)] per n-tile, cast to fp16
                for nt in range(8):
                    pv = setup_ps.tile([128, 512], f32, name=f"pv{nt}", tag="pv", bufs=2)
                    nc.tensor.matmul(pv[:, :], xf[:, nt * 128:(nt + 1) * 128],
                                     wT[:, 1024:1536], start=True, stop=True)
                    vdst = v_all[:, nt * 520:(nt + 1) * 520].rearrange(
                        "p (h c) -> p h c", c=65)[:, :, 0:64]
                    nc.scalar.copy(out=vdst, in_=pv.rearrange("p (h c) -> p h c", c=64))
                    ones_col = v_all[:, nt * 520:(nt + 1) * 520].rearrange(
                        "p (h c) -> p h c", c=65)[:, :, 64:65]
                    nc.vector.memset(ones_col, 1.0)

                # bias tables -> qc rows 64..127; matmuls write PSUM partitions
                # 64..95 / 96..127 (col strips 2/3) so copies stay aligned.
                # BH_T[y, i] = q_i . rel_h[y - y_i + 31]: y_i = i//32 constant on
                # contiguous 32-column groups. BW_T[x, i] = q_i . rel_w[x - x_i + 31]:
                # x_i = i%32 constant on stride-32 column groups.
                qc_h = qc.rearrange("p (h n) -> p h n", n=N)
                qc_hyx = qc.rearrange("p (h y x) -> p h y x", y=32, x=32)
                for g in range(32):
                    pb = setup_ps.tile([128, 256], f32, name=f"pb{g}", tag="pb")
                    nc.tensor.matmul(pb[64:96, :], relcat[:, g * 64:g * 64 + 32],
                                     qc_h[0:64, :, g * 32:(g + 1) * 32],
                                     start=True, stop=True, tile_position=(0, 64))
                    nc.scalar.copy(out=qc_h[64:96, :, g * 32:(g + 1) * 32],
                                   in_=pb[64:96, :].rearrange("p (h n) -> p h n", n=32))
                # BW goes through col strip 2 (quadrant 3 is buggy) + an SBUF
                # stage; the DMA does the partition shift 64..95 -> 96..127.
                for s in range(32):
                    pw = setup_ps.tile([128, 256], f32, name=f"pw{s}", tag="pw")
                    nc.tensor.matmul(pw[64:96, :], relcat[:, s * 64 + 32:s * 64 + 64],
                                     qc_hyx[0:64, :, :, s],
                                     start=True, stop=True, tile_position=(0, 64))
                    bw_stage = setup_sb.tile([96, 256], f32, name=f"bw_stage{s}",
                                             tag="bw_stage", bufs=4)
                    nc.scalar.copy(out=bw_stage[64:96, :], in_=pw[64:96, :])
                    nc.sync.dma_start(
                        out=qc_hyx[96:128, :, :, s],
                        in_=bw_stage[64:96, :].rearrange("p (h y) -> p h y", y=32))

            # bf16 copies for the stats pass (pre -m patch)
            nc.vector.tensor_copy(out=qcb, in_=qc)
            nc.vector.tensor_copy(out=kcb, in_=kc)

            # ---- attention ----
            with tc.tile_pool(name="stats_ps", bufs=1, space="PSUM") as stats_ps, \
                 tc.tile_pool(name="main_ps", bufs=2, space="PSUM") as main_ps, \
                 tc.tile_pool(name="av_ps", bufs=1, space="PSUM") as av_ps, \
                 tc.tile_pool(name="work", bufs=2) as work, \
                 tc.tile_pool(name="eT_pool", bufs=2) as eT_pool:

                for h in range(NUM_HEADS):
                    hs = h * N

                    # stats: normal layout sim+bias in bf16, row max
                    for it in range(8):
                        ps = stats_ps.tile([128, N], f32, name=f"ps_{h}_{it}", tag="ps")
                        nc.tensor.matmul(ps[:, 0:512], qcb[:, hs + it * 128:hs + (it + 1) * 128],
                                         kcb[:, hs:hs + 512], start=True, stop=True)
                        nc.tensor.matmul(ps[:, 512:1024], qcb[:, hs + it * 128:hs + (it + 1) * 128],
                                         kcb[:, hs + 512:hs + 1024], start=True, stop=True)
                        nc.vector.tensor_reduce(
                            out=negm_all[:, h * 8 + it:h * 8 + it + 1], in_=ps,
                            axis=mybir.AxisListType.X, op=mybir.AluOpType.max, negate=True)

                    # replicate -m across 32 partitions via DRAM round-trip
                    scr_m = dram.tile([1024], f32, name=f"scr_m{h}", tag="scr_m")
                    # write scratch as [t*128 + i] so the broadcast read is contiguous
                    dst_m = bass.AP(tensor=scr_m.tensor, offset=scr_m.offset,
                                    ap=[[1, 128], [128, 8]])
                    nc.sync.dma_start(out=dst_m, in_=negm_all[:, h * 8:h * 8 + 8])
                    # negm_rep must sit on partitions 64..95 to pair with qc[64:96]
                    negm_rep = work.tile([96, N], f32, name=f"negm_rep{h}", tag="negm_rep")
                    src_m = bass.AP(tensor=scr_m.tensor, offset=scr_m.offset,
                                    ap=[[0, 32], [1, 1024]])
                    nc.sync.dma_start(out=negm_rep[64:96, :], in_=src_m)
                    # patch: qc rows 64..95 (BH_T) += -m
                    nc.vector.tensor_add(out=qc[64:96, hs:hs + N],
                                         in0=qc[64:96, hs:hs + N], in1=negm_rep[64:96, :])

                    # main transposed pass: sim_T + bias - m -> exp -> e_T (fp16)
                    eTs = []
                    for jt in range(8):
                        pm = main_ps.tile([128, N], f32, name=f"pm_{h}_{jt}", tag="pm")
                        nc.tensor.matmul(pm[:, 0:512], kc[:, hs + jt * 128:hs + (jt + 1) * 128],
                                         qc[:, hs:hs + 512], start=True, stop=True)
                        nc.tensor.matmul(pm[:, 512:1024], kc[:, hs + jt * 128:hs + (jt + 1) * 128],
                                         qc[:, hs + 512:hs + 1024], start=True, stop=True)
                        eT = eT_pool.tile([128, N], fp16, name=f"eT{jt}", tag=f"eT{jt}")
                        nc.scalar.activation(out=eT, in_=pm,
                                             func=mybir.ActivationFunctionType.Exp)
                        eTs.append(eT)

                    # AV: out_T[d,i] (+ denominator row 64) accumulated over j-tiles
                    s_sb = work.tile([65, N], f32, name=f"s_sb{h}", tag="s_sb")
                    out_u = work.tile([64, N], f32, name=f"out_u{h}", tag="out_u")
                    for ih in range(2):
                        pa = av_ps.tile([65, 512], f32, name=f"pa_{h}_{ih}", tag="pa")
                        for jt in range(8):
                            nc.tensor.matmul(
                                pa[:, :],
                                v_all[:, jt * 520 + h * 65:jt * 520 + (h + 1) * 65],
                                eTs[jt][:, ih * 512:(ih + 1) * 512],
                                start=(jt == 0), stop=(jt == 7))
                        nc.scalar.copy(out=s_sb[64:65, ih * 512:(ih + 1) * 512],
                                       in_=pa[64:65, :])
                        nc.scalar.copy(out=out_u[:, ih * 512:(ih + 1) * 512], in_=pa[0:64, :])

                    # normalize: r = 1/s, replicate across 64 partitions, multiply
                    r_sb = work.tile([65, N], f32, name=f"r_sb{h}", tag="r_sb")
                    nc.vector.reciprocal(r_sb[64:65, :], s_sb[64:65, :])
                    scr_r = dram.tile([1024], f32, name=f"scr_r{h}", tag="scr_r")
                    nc.sync.dma_start(out=scr_r, in_=r_sb[64:65, :])
                    r_rep = work.tile([64, N], f32, name=f"r_rep{h}", tag="r_rep")
                    src_r = bass.AP(tensor=scr_r.tensor, offset=scr_r.offset,
                                    ap=[[0, 64], [1, 1024]])
                    nc.sync.dma_start(out=r_rep, in_=src_r)
                    out_sb = work.tile([64, N], f32, name=f"out_sb{h}", tag="out_sb")
                    nc.vector.tensor_mul(out=out_sb, in0=out_u, in1=r_rep)
                    nc.sync.dma_start(out=out_p[h * 64:(h + 1) * 64, :], in_=out_sb)

    nc.finalize()
    return nc


def _host_inputs(x, w_qkv, rel_h, rel_w):
    """Per-core input maps (core b gets batch b)."""
    x = np.asarray(x, dtype=np.float32)
    w_qkv = np.asarray(w_qkv, dtype=np.float32)
    rel_h = np.asarray(rel_h, dtype=np.float32)
    rel_w = np.asarray(rel_w, dtype=np.float32)

    wT = np.ascontiguousarray(w_qkv.T)  # [C, 1536]
    wT[:, 0:512] = wT[:, 0:512] * SCALE  # fold q scale

    relcat = np.empty((64, 32 * 64), dtype=np.float32)
    for g in range(32):
        relcat[:, g * 64:g * 64 + 32] = rel_h[31 - g:63 - g, :].T
        relcat[:, g * 64 + 32:g * 64 + 64] = rel_w[31 - g:63 - g, :].T

    oh = np.zeros((64, N), dtype=np.float32)
    j = np.arange(N)
    oh[j // W, j] = 1.0
    oh[32 + (j % W), j] = 1.0

    in_maps = []
    for b in range(B):
        in_maps.append({
            "xf": np.ascontiguousarray(x[b].reshape(C, N)),
            "wT": wT,
            "relcat": relcat,
            "oh": oh,
        })
    return in_maps


def kernel(x, w_qkv, rel_h, rel_w):
    from concourse.bass_utils import run_bass_kernel_spmd

    if "nc" not in _cache:
        _cache["nc"] = _build_program()
    nc = _cache["nc"]

    in_maps = _host_inputs(x, w_qkv, rel_h, rel_w)
    res = run_bass_kernel_spmd(nc, in_maps, list(range(B)),
                               trace=bool(int(os.environ.get("KERNEL_TRACE", "0"))))
    _cache["last_results"] = res
    out = np.stack([res.results[b]["out"] for b in range(B)], axis=0)  # [8, 512, 1024]
    return out.reshape(B, NUM_HEADS * DIM_HEAD, H, W)

